# revision 1
# baseline (speedup 1.0000x reference)
"""Trainium2 Bass kernel for nn_ContinousNormalizingFlowRHS.

Computes, for z in R^{B x Z} and scalar time t:
  h0 = tanh(W1*t + B1); h1 = tanh(einsum('knm,km->kn', W2, h0) + B2)
  w_in  = (W3_win  @ h1[0] + b3_win ).reshape(F, Z)
  w_out = (W3_wout @ h1[1] + b3_wout).reshape(F, Z)
  b     =  W3_b    @ h1[2] + b3_b
  gate  = sigmoid(W3_gate @ h1[3] + b3_gate)
  h = tanh(z @ w_in.T + b); dz = (h*gate) @ w_out / F
  trace = ((1-h^2)*gate) @ (sum(w_in*w_out,1)) / F
  out = concat([dz, -trace[:,None]], -1)

Strategy (8 NeuronCores, single SPMD launch):
  The dominant cost is streaming W3_win/W3_wout (268 MB each) for the
  matvecs, so those are sharded row-wise across the 8 cores (F-sharding).
  Each core's matvec work is further split between the PE (transposed
  bf16 slices as stationary weights, h1 column as the moving operand)
  and the DVE (natural-layout slices, multiply by a partition-broadcast
  h1 then reduce along the free axis), so neither engine is the
  bottleneck and the HBM stream rate binds.  Each core then runs the
  batch matmuls for the FULL batch against its local f-slice, producing
  partial dz/trace sums.  Two pipelined ReduceScatter(add) ops complete
  the sum over F and hand each core its own batch shard of the output.
"""

import sys
import types
import numpy as np
import ml_dtypes

BF = ml_dtypes.bfloat16

# problem sizes (hardcoded per contract)
Z = 128
N = 256
F = 2048
B = 8192
N_CORES = 8

PE_COLS = 128       # per matrix: f-columns computed on the PE (rest on DVE)
CHUNK_R = 4096      # W3 rows per streamed PE chunk ([128, 4096] bf16 tiles)
DVE_CC = 16         # f-columns per DVE chunk (2048 rows)
BC = 512            # batch columns per stage-B chunk (one PSUM bank)


def _ensure_ntff_hook():
    """run_bass_kernel_spmd(trace=True) under axon needs antenv.axon_hooks."""
    if 'antenv.axon_hooks' in sys.modules:
        return
    try:
        from trn_agent_boot.trn_boot import _ntff_profile_via_ctypes
        hook = _ntff_profile_via_ctypes('/opt/axon/libaxon_pjrt.so')
    except Exception:
        hook = None
    try:
        import antenv
    except Exception:
        return
    mod = types.ModuleType('antenv.axon_hooks')
    mod.get_axon_ntff_profile_hook = lambda: hook
    mod.set_axon_ntff_profile_hook = lambda h: None
    sys.modules['antenv.axon_hooks'] = mod
    antenv.axon_hooks = mod


def build_module(n_cores=N_CORES, b=B, f=F, pe_cols=PE_COLS, chunk_r=CHUNK_R,
                 bc=BC, debug=False):
    """Build the Bass module (SPMD program, one per core)."""
    import concourse.tile as tile
    from concourse import bacc, mybir

    F32 = mybir.dt.float32
    BF16 = mybir.dt.bfloat16
    ADD = mybir.AluOpType.add

    fl = f // n_cores            # local f count
    nfb = fl // 128              # local f blocks of 128
    rows_pe = pe_cols * 128      # rows of W3 handled by the PE
    dve_cols = fl - pe_cols
    rows_dve = dve_cols * 128
    n_pe_chunks = rows_pe // chunk_r
    rpc = chunk_r // 128         # w columns produced per PE chunk
    dcc = DVE_CC                 # f-columns per DVE chunk
    n_dve_chunks = dve_cols // dcc
    bl = b // n_cores            # output batch shard
    hw = bl // 2                 # reduce-scatter half width
    assert rows_pe % chunk_r == 0 and dve_cols % dcc == 0
    assert hw % bc == 0

    nc = bacc.Bacc("TRN2", target_bir_lowering=False, debug=debug,
                   num_devices=n_cores)

    def inp(name, shape, dt):
        return nc.dram_tensor(name, shape, dt, kind="ExternalInput").ap()

    t_ap = inp("t", [1, 1], F32)
    w1_ap = inp("w1c", [128, 8], F32)
    b1_ap = inp("b1c", [128, 8], F32)
    b2_ap = inp("b2c", [128, 8], F32)
    w2t_ap = inp("w2tc", [128, 2048], BF16)
    w3winT_ap = inp("w3winT_sl", [N, rows_pe], BF16)
    w3woutT_ap = inp("w3woutT_sl", [N, rows_pe], BF16)
    w3winN_ap = inp("w3winN_sl", [rows_dve // (dcc * 128) * 128, dcc * N], BF16)
    w3woutN_ap = inp("w3woutN_sl", [rows_dve // (dcc * 128) * 128, dcc * N], BF16)
    b3win_ap = inp("b3win_c", [128, fl], F32)
    b3wout_ap = inp("b3wout_c", [128, fl], F32)
    w3bT_ap = inp("w3bT_sl", [N, fl], BF16)
    w3gateT_ap = inp("w3gateT_sl", [N, fl], BF16)
    b3b_ap = inp("b3b_c", [128, nfb], F32)
    b3gate_ap = inp("b3gate_c", [128, nfb], F32)
    zt_ap = inp("ztb", [128, b], BF16)
    eye_ap = inp("eyeb", [128, 128], BF16)
    out_ap = nc.dram_tensor("out", [Z + 1, bl], F32, kind="ExternalOutput").ap()

    with tile.TileContext(nc) as tc:
        with tc.tile_pool(name="persist", bufs=1) as pp, \
             tc.tile_pool(name="stream", bufs=4) as sp, \
             tc.tile_pool(name="work", bufs=3) as wp, \
             tc.tile_pool(name="ps_h", bufs=2, space="PSUM") as ps_h, \
             tc.tile_pool(name="ps_dz", bufs=2, space="PSUM") as ps_dz, \
             tc.tile_pool(name="ps_t2", bufs=2, space="PSUM") as ps_t2, \
             tc.tile_pool(name="ps_prep", bufs=2, space="PSUM") as ps_prep, \
             tc.tile_pool(name="dram", bufs=1, space="DRAM") as dp:

            # ---- parameter nets (tiny) ----------------------------------
            t_bc = pp.tile([128, 1], F32, tag="tbc")
            nc.gpsimd.dma_start(t_bc[:], t_ap.broadcast_to([128, 1]))
            w1_sb = pp.tile([128, 8], F32, tag="w1")
            b1_sb = pp.tile([128, 8], F32, tag="b1")
            b2_sb = pp.tile([128, 8], F32, tag="b2")
            w2t_sb = pp.tile([128, 2048], BF16, tag="w2t")
            nc.gpsimd.dma_start(w1_sb[:], w1_ap[:])
            nc.gpsimd.dma_start(b1_sb[:], b1_ap[:])
            nc.gpsimd.dma_start(b2_sb[:], b2_ap[:])
            nc.gpsimd.dma_start(w2t_sb[:], w2t_ap[:])

            h0pre = pp.tile([128, 8], F32, tag="h0pre")
            nc.vector.tensor_scalar_mul(h0pre[:], w1_sb[:], t_bc[:, 0:1])
            nc.vector.tensor_add(h0pre[:], h0pre[:], b1_sb[:])
            h0_sb = pp.tile([128, 8], BF16, tag="h0")
            nc.scalar.activation(h0_sb[:], h0pre[:],
                                 mybir.ActivationFunctionType.Tanh)

            ps_h1 = ps_prep.tile([128, 8], F32, tag="prep")
            for k4 in range(4):
                for nb in range(2):
                    c = k4 * 2 + nb
                    for mb in range(2):
                        lhs = w2t_sb[:, k4 * 512 + mb * 256 + nb * 128:
                                     k4 * 512 + mb * 256 + nb * 128 + 128]
                        nc.tensor.matmul(ps_h1[:, c:c + 1], lhs,
                                         h0_sb[:, k4 * 2 + mb:k4 * 2 + mb + 1],
                                         start=(mb == 0), stop=(mb == 1))
            h1pre = pp.tile([128, 8], F32, tag="h1pre")
            h1_sb = pp.tile([128, 8], BF16, tag="h1")
            nc.vector.tensor_add(h1pre[:], ps_h1[:], b2_sb[:])
            nc.scalar.activation(h1_sb[:], h1pre[:],
                                 mybir.ActivationFunctionType.Tanh)
            # h1 -> DRAM in (net, n) order, then broadcast-load nets 0/1
            # replicated across partitions AND repeated dcc times along the
            # free dim (so the DVE multiply runs chunk-granular).
            h1_dram = dp.tile([8, 128], BF16, tag="h1d")
            nc.gpsimd.dma_start(h1_dram.rearrange("c n -> n c"), h1_sb[:])
            h1b = []
            for k4 in range(2):
                hb = pp.tile([128, dcc * N], BF16, tag=f"h1b{k4}")
                src = h1_dram.rearrange("c n -> (c n)")[k4 * N:(k4 + 1) * N]
                src = src.unsqueeze(0).unsqueeze(0)
                nc.gpsimd.dma_start(hb[:], src.broadcast_to([128, dcc, N]))
                h1b.append(hb)

            # ---- phase 1: sharded matvecs, split across PE and DVE ------
            b3win_sb = pp.tile([128, fl], F32, tag="b3win")
            b3wout_sb = pp.tile([128, fl], F32, tag="b3wout")
            nc.scalar.dma_start(b3win_sb[:], b3win_ap[:])
            nc.scalar.dma_start(b3wout_sb[:], b3wout_ap[:])

            w_inT_bf = pp.tile([128, fl], BF16, tag="winT")
            w_outT_bf = pp.tile([128, fl], BF16, tag="woutT")

            # PE part: columns [0, pe_cols) of each matrix
            for w3T_ap, bias_sb, dst, net in ((w3winT_ap, b3win_sb, w_inT_bf, 0),
                                              (w3woutT_ap, b3wout_sb, w_outT_bf, 1)):
                for c in range(n_pe_chunks):
                    tiles = []
                    for nb in range(2):
                        w3t = sp.tile([128, chunk_r], BF16, tag="w3chunk")
                        nc.sync.dma_start(
                            w3t[:], w3T_ap[nb * 128:(nb + 1) * 128,
                                           c * chunk_r:(c + 1) * chunk_r])
                        tiles.append(w3t)
                    pw = ps_prep.tile([128, rpc], F32, tag="prep")
                    for a in range(rpc):
                        for nb in range(2):
                            nc.tensor.matmul(
                                pw[:, a:a + 1],
                                tiles[nb][:, a * 128:(a + 1) * 128],
                                h1_sb[:, net * 2 + nb:net * 2 + nb + 1],
                                start=(nb == 0), stop=(nb == 1))
                    nc.vector.tensor_add(dst[:, c * rpc:(c + 1) * rpc], pw[:],
                                         bias_sb[:, c * rpc:(c + 1) * rpc])

            # DVE part: columns [pe_cols, fl) of each matrix, one chunk-wide
            # multiply + one 3-D reduce per dcc columns.
            for w3N_ap, bias_sb, dst, net in ((w3winN_ap, b3win_sb, w_inT_bf, 0),
                                              (w3woutN_ap, b3wout_sb, w_outT_bf, 1)):
                acc = pp.tile([128, max(dve_cols, 1)], F32, tag=f"dacc{net}")
                for c in range(n_dve_chunks):
                    w3n = sp.tile([128, dcc * N], BF16, tag="w3nat")
                    nc.scalar.dma_start(w3n[:],
                                        w3N_ap[c * 128:(c + 1) * 128, :])
                    prod = wp.tile([128, dcc * N], BF16, tag="prod")
                    nc.vector.tensor_mul(prod[:], w3n[:], h1b[net][:])
                    nc.vector.tensor_reduce(
                        acc[:, c * dcc:(c + 1) * dcc],
                        prod.rearrange("p (a n) -> p a n", a=dcc),
                        mybir.AxisListType.X, ADD)
                if dve_cols:
                    nc.vector.tensor_add(dst[:, pe_cols:fl], acc[:, 0:dve_cols],
                                         bias_sb[:, pe_cols:fl])

            # heads: b and gate (psum [f, fb] columns)
            b3b_sb = pp.tile([128, nfb], F32, tag="b3b")
            b3gate_sb = pp.tile([128, nfb], F32, tag="b3gate")
            nc.gpsimd.dma_start(b3b_sb[:], b3b_ap[:])
            nc.gpsimd.dma_start(b3gate_sb[:], b3gate_ap[:])
            b_sb = pp.tile([128, nfb], F32, tag="bh")
            gate_sb = pp.tile([128, nfb], F32, tag="gate")
            gpre = pp.tile([128, nfb], F32, tag="gpre")
            for w3hT_ap, bias_sb, dst, net in ((w3bT_ap, b3b_sb, b_sb, 2),
                                               (w3gateT_ap, b3gate_sb, gpre, 3)):
                w3ht = sp.tile([128, 2 * fl], BF16, tag="w3head")
                nc.scalar.dma_start(
                    w3ht[:], w3hT_ap.rearrange("(nb p) fl -> p nb fl", p=128))
                phd = ps_prep.tile([128, nfb], F32, tag="prep")
                for a in range(nfb):
                    for nb in range(2):
                        nc.tensor.matmul(
                            phd[:, a:a + 1],
                            w3ht[:, nb * fl + a * 128:nb * fl + (a + 1) * 128],
                            h1_sb[:, net * 2 + nb:net * 2 + nb + 1],
                            start=(nb == 0), stop=(nb == 1))
                nc.vector.tensor_add(dst[:], phd[:], bias_sb[:])
            nc.scalar.activation(gate_sb[:], gpre[:],
                                 mybir.ActivationFunctionType.Sigmoid)

            # ---- stage-B constants --------------------------------------
            zt_sb = pp.tile([128, b], BF16, tag="zt")
            nc.scalar.dma_start(zt_sb[:], zt_ap[:])
            eye_sb = pp.tile([128, 128], BF16, tag="eye")
            nc.gpsimd.dma_start(eye_sb[:], eye_ap[:])

            # transpose w_in/w_out to [f, z]; fold gate into w_out
            w_outg = pp.tile([128, nfb * 128], BF16, tag="woutg")
            w_in_fz = pp.tile([128, nfb * 128], BF16, tag="winfz")
            sg = pp.tile([128, nfb], F32, tag="sg")
            for fb in range(nfb):
                ptr = ps_prep.tile([128, 128], BF16, tag="prep")
                nc.tensor.transpose(ptr[:], w_outT_bf[:, fb * 128:(fb + 1) * 128],
                                    eye_sb[:])
                nc.vector.tensor_scalar_mul(w_outg[:, fb * 128:(fb + 1) * 128],
                                            ptr[:], gate_sb[:, fb:fb + 1])
                pti = ps_prep.tile([128, 128], BF16, tag="prep")
                nc.tensor.transpose(pti[:], w_inT_bf[:, fb * 128:(fb + 1) * 128],
                                    eye_sb[:])
                nc.vector.tensor_copy(w_in_fz[:, fb * 128:(fb + 1) * 128], pti[:])
                # s' = sum_z w_in[f,z] * w_out[f,z] * gate[f]
                prod = wp.tile([128, 128], F32, tag="sprod")
                nc.vector.tensor_mul(prod[:], w_in_fz[:, fb * 128:(fb + 1) * 128],
                                     w_outg[:, fb * 128:(fb + 1) * 128])
                nc.vector.tensor_reduce(sg[:, fb:fb + 1], prod[:],
                                        mybir.AxisListType.X, ADD)
            sg_bf = pp.tile([128, nfb], BF16, tag="sgbf")
            nc.vector.tensor_copy(sg_bf[:], sg[:])
            # cneg = -sum_f s' / F
            csum = pp.tile([1, 1], F32, tag="csum")
            nc.gpsimd.tensor_reduce(csum[:], sg[:], mybir.AxisListType.XYZWC, ADD)
            cneg = pp.tile([1, 1], F32, tag="cneg")
            nc.scalar.mul(cneg[:], csum[:], -1.0 / f)

            # ---- stage B: batch matmuls over local f slice --------------
            # half h of every core's [Z+1, bl] output reduces in its own
            # ReduceScatter so the first one overlaps remaining compute.
            cc_in = [dp.tile([n_cores, Z, hw], BF16, tag=f"ccin{h}",
                             name=f"ccin{h}") for h in range(2)]
            cc_out = [dp.tile([Z, hw], BF16, tag=f"ccout{h}",
                              name=f"ccout{h}") for h in range(2)]
            cc_tr_in = dp.tile([n_cores, bl], F32, tag="cctri", name="cctri")
            cc_tr_out = dp.tile([1, bl], F32, tag="cctro", name="cctro")
            for half in range(2):
                for kk in range(n_cores):
                    for j in range(hw // bc):
                        g0 = kk * bl + half * hw + j * bc
                        pdz = ps_dz.tile([128, bc], F32, tag="pdz")
                        pt2 = ps_t2.tile([1, bc], F32, tag="pt2")
                        for fb in range(nfb):
                            ph = ps_h.tile([128, bc], F32, tag="ph")
                            nc.tensor.matmul(ph[:],
                                             w_inT_bf[:, fb * 128:(fb + 1) * 128],
                                             zt_sb[:, g0:g0 + bc],
                                             start=True, stop=True)
                            h_bf = wp.tile([128, bc], BF16, tag="hbf")
                            nc.scalar.activation(
                                h_bf[:], ph[:],
                                mybir.ActivationFunctionType.Tanh,
                                bias=b_sb[:, fb:fb + 1])
                            h2_bf = wp.tile([128, bc], BF16, tag="h2bf")
                            nc.vector.tensor_mul(h2_bf[:], h_bf[:], h_bf[:])
                            nc.tensor.matmul(pdz[:],
                                             w_outg[:, fb * 128:(fb + 1) * 128],
                                             h_bf[:],
                                             start=(fb == 0), stop=(fb == nfb - 1))
                            nc.tensor.matmul(pt2[:], sg_bf[:, fb:fb + 1], h2_bf[:],
                                             start=(fb == 0), stop=(fb == nfb - 1))
                        dz_sb = wp.tile([128, bc], BF16, tag="dzsb")
                        nc.scalar.mul(dz_sb[:], pdz[:], 1.0 / f)
                        tr_sb = wp.tile([1, bc], F32, tag="trsb")
                        nc.scalar.activation(
                            tr_sb[:], pt2[:],
                            mybir.ActivationFunctionType.Identity,
                            bias=cneg[0:1, 0:1], scale=1.0 / f)
                        off = j * bc
                        nc.sync.dma_start(cc_in[half][kk, :, off:off + bc],
                                          dz_sb[:])
                        nc.sync.dma_start(
                            cc_tr_in[kk, half * hw + off:half * hw + off + bc]
                            .unsqueeze(0), tr_sb[:])
                nc.gpsimd.collective_compute(
                    "ReduceScatter", ADD,
                    replica_groups=[list(range(n_cores))],
                    ins=[cc_in[half].opt()], outs=[cc_out[half].opt()])
                nc.gpsimd.dma_start(out_ap[0:Z, half * hw:(half + 1) * hw],
                                    cc_out[half][:])
            nc.gpsimd.collective_compute(
                "ReduceScatter", ADD,
                replica_groups=[list(range(n_cores))],
                ins=[cc_tr_in.opt()], outs=[cc_tr_out.opt()])
            nc.gpsimd.dma_start(out_ap[Z:Z + 1, :], cc_tr_out[:])

    nc.compile()
    return nc


def host_prep(t, z_and_logpz, W1, B1, W2, B2, W3_win, b3_win,
              W3_wout, b3_wout, W3_b, b3_b, W3_gate, b3_gate,
              n_cores=N_CORES, b=B, f=F, pe_cols=PE_COLS):
    """Shard + lay out the numpy inputs into per-core in_maps."""
    fl = f // n_cores
    nfb = fl // 128
    rows = fl * Z
    rows_pe = pe_cols * 128

    dcc = DVE_CC

    def pack_nat(x):  # [rows_dve, N] -> [nch*128, dcc*N], partition-contiguous
        nch = x.shape[0] // (dcc * 128)
        return np.ascontiguousarray(
            x.reshape(nch, dcc, 128, N).transpose(0, 2, 1, 3)
            .reshape(nch * 128, dcc * N))

    def col8(x):  # [4, 256] -> [128, 8] with col = k*2 + nb
        return np.ascontiguousarray(
            np.asarray(x, np.float32).reshape(4, 2, 128).transpose(2, 0, 1)
            .reshape(128, 8))

    t_in = np.asarray(t, np.float32).reshape(1, 1)
    w1c = col8(np.asarray(W1, np.float32)[:, :, 0])
    b1c = col8(B1)
    b2c = col8(B2)
    # lhsT tile for h1 net: [m128, (k4, mb, n)] = W2[k4, n, mb*128+m128]
    w2tc = np.ascontiguousarray(
        np.asarray(W2, np.float32).transpose(0, 2, 1)        # [k, m, n]
        .reshape(4, 2, 128, 256).transpose(2, 0, 1, 3).reshape(128, 2048)).astype(BF)
    w3win_bf = np.asarray(W3_win, np.float32).astype(BF)
    w3wout_bf = np.asarray(W3_wout, np.float32).astype(BF)
    w3b_bf = np.asarray(W3_b, np.float32).astype(BF)
    w3gate_bf = np.asarray(W3_gate, np.float32).astype(BF)
    b3win = np.asarray(b3_win, np.float32)
    b3wout = np.asarray(b3_wout, np.float32)
    b3b = np.asarray(b3_b, np.float32)
    b3gate = np.asarray(b3_gate, np.float32)
    z = np.asarray(z_and_logpz, np.float32)[:, :Z]
    ztb = np.ascontiguousarray(z.T).astype(BF)
    eye = np.eye(128, dtype=np.float32).astype(BF)

    in_maps = []
    for k in range(n_cores):
        r0 = k * rows
        f0 = k * fl
        in_maps.append({
            "t": t_in, "w1c": w1c, "b1c": b1c, "b2c": b2c, "w2tc": w2tc,
            "w3winT_sl": np.ascontiguousarray(w3win_bf[r0:r0 + rows_pe].T),
            "w3woutT_sl": np.ascontiguousarray(w3wout_bf[r0:r0 + rows_pe].T),
            "w3winN_sl": pack_nat(w3win_bf[r0 + rows_pe:r0 + rows]),
            "w3woutN_sl": pack_nat(w3wout_bf[r0 + rows_pe:r0 + rows]),
            "b3win_c": np.ascontiguousarray(
                b3win[r0:r0 + rows].reshape(fl, 128).T),
            "b3wout_c": np.ascontiguousarray(
                b3wout[r0:r0 + rows].reshape(fl, 128).T),
            "w3bT_sl": np.ascontiguousarray(w3b_bf[f0:f0 + fl].T),
            "w3gateT_sl": np.ascontiguousarray(w3gate_bf[f0:f0 + fl].T),
            "b3b_c": np.ascontiguousarray(b3b[f0:f0 + fl].reshape(nfb, 128).T),
            "b3gate_c": np.ascontiguousarray(
                b3gate[f0:f0 + fl].reshape(nfb, 128).T),
            "ztb": ztb, "eyeb": eye,
        })
    return in_maps


_NC_CACHE = {}


def kernel(**inputs) -> np.ndarray:
    _ensure_ntff_hook()
    from concourse import bass_utils

    key = "full"
    if key not in _NC_CACHE:
        _NC_CACHE[key] = build_module()
    nc = _NC_CACHE[key]

    in_maps = host_prep(**inputs)
    res = bass_utils.run_bass_kernel_spmd(nc, in_maps, list(range(N_CORES)))
    bl = B // N_CORES
    out = np.empty((B, Z + 1), np.float32)
    for k in range(N_CORES):
        out[k * bl:(k + 1) * bl, :] = res.results[k]["out"].T
    return out



# revision 6
# speedup vs baseline: 1.3200x; 1.3200x over previous
"""Trainium2 Bass kernel for nn_ContinousNormalizingFlowRHS.

Computes, for z in R^{B x Z} and scalar time t:
  h0 = tanh(W1*t + B1); h1 = tanh(einsum('knm,km->kn', W2, h0) + B2)
  w_in  = (W3_win  @ h1[0] + b3_win ).reshape(F, Z)
  w_out = (W3_wout @ h1[1] + b3_wout).reshape(F, Z)
  b     =  W3_b    @ h1[2] + b3_b
  gate  = sigmoid(W3_gate @ h1[3] + b3_gate)
  h = tanh(z @ w_in.T + b); dz = (h*gate) @ w_out / F
  trace = ((1-h^2)*gate) @ (sum(w_in*w_out,1)) / F
  out = concat([dz, -trace[:,None]], -1)

Strategy (8 NeuronCores, single SPMD launch):
  Phase 1 (f-sharded): the dominant cost is streaming W3_win/W3_wout
  (134 MB each in bf16), sharded row-wise across the 8 cores.  Each
  core's matvec work is split between the PE (transposed bf16 slices
  as stationary weights, h1 column as the moving operand) and the DVE
  (natural-layout slices, multiply by a partition-broadcast h1 then
  reduce along the free axis), so the HBM stream rate binds.
  Each core then packs its local slice of (w_inT, gate*w_out, sg, b)
  into a ~129 KB blob; one small AllGather replicates all slices.
  Stage B (batch-sharded): each core runs the batch matmuls for its
  OWN B/8 shard against the FULL F, accumulating dz and the trace in
  fp32 PSUM across all 16 f-blocks, and writes its final [Z+1, B/8]
  output directly -- no end-of-kernel collective.
"""

import sys
import types
import numpy as np
import ml_dtypes

BF = ml_dtypes.bfloat16

# problem sizes (hardcoded per contract)
Z = 128
N = 256
F = 2048
B = 8192
N_CORES = 8

PE_COLS = 128       # per matrix: local f-columns computed on the PE (rest DVE)
CHUNK_R = 4096      # W3 rows per streamed PE chunk ([128, 4096] bf16 tiles)
DVE_CC = 16         # f-columns per DVE chunk (2048 rows)
BC = 512            # batch columns per stage-B chunk (one PSUM bank)


def _ensure_ntff_hook():
    """run_bass_kernel_spmd(trace=True) under axon needs antenv.axon_hooks."""
    if 'antenv.axon_hooks' in sys.modules:
        return
    try:
        from trn_agent_boot.trn_boot import _ntff_profile_via_ctypes
        hook = _ntff_profile_via_ctypes('/opt/axon/libaxon_pjrt.so')
    except Exception:
        hook = None
    try:
        import antenv
    except Exception:
        return
    mod = types.ModuleType('antenv.axon_hooks')
    mod.get_axon_ntff_profile_hook = lambda: hook
    mod.set_axon_ntff_profile_hook = lambda h: None
    sys.modules['antenv.axon_hooks'] = mod
    antenv.axon_hooks = mod


def build_module(n_cores=N_CORES, b=B, f=F, pe_cols=PE_COLS, chunk_r=CHUNK_R,
                 bc=BC, debug=False):
    """Build the Bass module (SPMD program, one per core)."""
    import concourse.tile as tile
    from concourse import bacc, mybir

    F32 = mybir.dt.float32
    BF16 = mybir.dt.bfloat16
    ADD = mybir.AluOpType.add

    fl = f // n_cores            # local f count
    nfb = fl // 128              # local f blocks of 128
    nfb_g = f // 128             # global f blocks of 128
    rows_pe = pe_cols * 128      # rows of W3 handled by the PE
    dve_cols = fl - pe_cols
    rows_dve = dve_cols * 128
    n_pe_chunks = rows_pe // chunk_r
    rpc = chunk_r // 128         # w columns produced per PE chunk
    dcc = DVE_CC                 # f-columns per DVE chunk
    n_dve_chunks = dve_cols // dcc
    bl = b // n_cores            # per-core batch shard
    nbc = bl // bc               # stage-B batch chunks
    assert rows_pe % chunk_r == 0 and dve_cols % dcc == 0

    # blob layout (bf16 elements): w_inT (z,f) | w_outg (fb,f,z) | sg | b
    SZ_A = 128 * fl
    SZ_B = 128 * fl
    SZ_C = fl
    SZ_D = fl
    BLOB = SZ_A + SZ_B + SZ_C + SZ_D
    OF_B, OF_C, OF_D = SZ_A, SZ_A + SZ_B, SZ_A + SZ_B + SZ_C

    nc = bacc.Bacc("TRN2", target_bir_lowering=False, debug=debug,
                   num_devices=n_cores)

    def inp(name, shape, dt):
        return nc.dram_tensor(name, shape, dt, kind="ExternalInput").ap()

    t_ap = inp("t", [128, 1], F32)                  # t replicated
    par_ap = inp("parc", [128, 24], F32)            # w1c | b1c | b2c
    w2t_ap = inp("w2tc", [128, 2048], BF16)
    w3winT_ap = inp("w3winT_sl", [N, rows_pe], BF16)
    w3woutT_ap = inp("w3woutT_sl", [N, rows_pe], BF16)
    w3winN_ap = inp("w3winN_sl", [rows_dve // (dcc * 128) * 128, dcc * N], BF16)
    w3woutN_ap = inp("w3woutN_sl", [rows_dve // (dcc * 128) * 128, dcc * N], BF16)
    b3win_ap = inp("b3win_c", [128, fl], F32)
    b3wout_ap = inp("b3wout_c", [128, fl], F32)
    w3bT_ap = inp("w3bT_sl", [N, fl], BF16)
    w3gateT_ap = inp("w3gateT_sl", [N, fl], BF16)
    b3b_ap = inp("b3b_c", [128, nfb], F32)
    b3gate_ap = inp("b3gate_c", [128, nfb], F32)
    zt_ap = inp("ztb_sl", [128, bl], BF16)          # own batch shard only
    eye_ap = inp("eyeb", [128, 128], BF16)
    out_ap = nc.dram_tensor("out", [Z + 1, bl], F32, kind="ExternalOutput").ap()

    with tile.TileContext(nc) as tc:
        with tc.tile_pool(name="persist", bufs=1) as pp, \
             tc.tile_pool(name="sp_pe", bufs=3) as sp_pe, \
             tc.tile_pool(name="sp_dve", bufs=3) as sp_dve, \
             tc.tile_pool(name="work", bufs=3) as wp, \
             tc.tile_pool(name="hbuf", bufs=4) as hp, \
             tc.tile_pool(name="h2buf", bufs=4) as h2p, \
             tc.tile_pool(name="ps_h", bufs=2, space="PSUM") as ps_h, \
             tc.tile_pool(name="ps_dz", bufs=2, space="PSUM") as ps_dz, \
             tc.tile_pool(name="ps_t2", bufs=2, space="PSUM") as ps_t2, \
             tc.tile_pool(name="ps_prep", bufs=2, space="PSUM") as ps_prep, \
             tc.tile_pool(name="dram", bufs=1, space="DRAM") as dp:

            # ---- parameter nets (tiny) ----------------------------------
            par_sb = pp.tile([128, 24], F32, tag="parc")
            nc.gpsimd.dma_start(par_sb[:], par_ap[:])
            t_sb = pp.tile([128, 1], F32, tag="tbc")
            nc.gpsimd.dma_start(t_sb[:], t_ap[:])
            w2t_sb = pp.tile([128, 2048], BF16, tag="w2t")
            nc.gpsimd.dma_start(w2t_sb[:], w2t_ap[:])

            h0pre = pp.tile([128, 8], F32, tag="h0pre")
            nc.vector.tensor_scalar_mul(h0pre[:], par_sb[:, 0:8], t_sb[:, 0:1])
            nc.vector.tensor_add(h0pre[:], h0pre[:], par_sb[:, 8:16])
            h0_sb = pp.tile([128, 8], BF16, tag="h0")
            nc.scalar.activation(h0_sb[:], h0pre[:],
                                 mybir.ActivationFunctionType.Tanh)

            ps_h1 = ps_prep.tile([128, 8], F32, tag="prep")
            for k4 in range(4):
                for nb in range(2):
                    c = k4 * 2 + nb
                    for mb in range(2):
                        lhs = w2t_sb[:, k4 * 512 + mb * 256 + nb * 128:
                                     k4 * 512 + mb * 256 + nb * 128 + 128]
                        nc.tensor.matmul(ps_h1[:, c:c + 1], lhs,
                                         h0_sb[:, k4 * 2 + mb:k4 * 2 + mb + 1],
                                         start=(mb == 0), stop=(mb == 1))
            h1pre = pp.tile([128, 8], F32, tag="h1pre")
            h1_sb = pp.tile([128, 8], BF16, tag="h1")
            nc.vector.tensor_add(h1pre[:], ps_h1[:], par_sb[:, 16:24])
            nc.scalar.activation(h1_sb[:], h1pre[:],
                                 mybir.ActivationFunctionType.Tanh)
            # h1 -> DRAM in (net, n) order, then broadcast-load nets 0/1
            # replicated across partitions AND repeated dcc times along the
            # free dim (so the DVE multiply runs chunk-granular).
            h1_dram = dp.tile([8, 128], BF16, tag="h1d")
            nc.gpsimd.dma_start(h1_dram.rearrange("c n -> n c"), h1_sb[:])
            h1b = []
            for k4 in range(2):
                hb = pp.tile([128, dcc * N], BF16, tag=f"h1b{k4}")
                src = h1_dram.rearrange("c n -> (c n)")[k4 * N:(k4 + 1) * N]
                src = src.unsqueeze(0).unsqueeze(0)
                nc.gpsimd.dma_start(hb[:], src.broadcast_to([128, dcc, N]))
                h1b.append(hb)

            # stage-B constants, loaded early
            zt_sb = pp.tile([128, bl], BF16, tag="zt")
            nc.scalar.dma_start(zt_sb[:], zt_ap[:])
            eye_sb = pp.tile([128, 128], BF16, tag="eye")
            nc.gpsimd.dma_start(eye_sb[:], eye_ap[:])
            b3win_sb = pp.tile([128, fl], F32, tag="b3win")
            b3wout_sb = pp.tile([128, fl], F32, tag="b3wout")
            nc.scalar.dma_start(b3win_sb[:], b3win_ap[:])
            nc.scalar.dma_start(b3wout_sb[:], b3wout_ap[:])

            # ---- phase 1: sharded matvecs, split across PE and DVE ------
            w_inT_bf = pp.tile([128, fl], BF16, tag="winT")
            w_outT_bf = pp.tile([128, fl], BF16, tag="woutT")

            # PE part: local columns [0, pe_cols) of each matrix
            for w3T_ap, bias_sb, dst, net in ((w3winT_ap, b3win_sb, w_inT_bf, 0),
                                              (w3woutT_ap, b3wout_sb, w_outT_bf, 1)):
                for c in range(n_pe_chunks):
                    tiles = []
                    for nb in range(2):
                        w3t = sp_pe.tile([128, chunk_r], BF16, tag="w3chunk")
                        nc.sync.dma_start(
                            w3t[:], w3T_ap[nb * 128:(nb + 1) * 128,
                                           c * chunk_r:(c + 1) * chunk_r])
                        tiles.append(w3t)
                    pw = ps_prep.tile([128, rpc], F32, tag="prep")
                    for a in range(rpc):
                        for nb in range(2):
                            nc.tensor.matmul(
                                pw[:, a:a + 1],
                                tiles[nb][:, a * 128:(a + 1) * 128],
                                h1_sb[:, net * 2 + nb:net * 2 + nb + 1],
                                start=(nb == 0), stop=(nb == 1))
                    nc.vector.tensor_add(dst[:, c * rpc:(c + 1) * rpc], pw[:],
                                         bias_sb[:, c * rpc:(c + 1) * rpc])

            # DVE part: local columns [pe_cols, fl) of each matrix
            for w3N_ap, bias_sb, dst, net in ((w3winN_ap, b3win_sb, w_inT_bf, 0),
                                              (w3woutN_ap, b3wout_sb, w_outT_bf, 1)):
                acc = pp.tile([128, max(dve_cols, 1)], F32, tag=f"dacc{net}")
                for c in range(n_dve_chunks):
                    w3n = sp_dve.tile([128, dcc * N], BF16, tag="w3nat")
                    nc.scalar.dma_start(w3n[:],
                                        w3N_ap[c * 128:(c + 1) * 128, :])
                    prod = wp.tile([128, dcc * N], BF16, tag="prod")
                    nc.vector.tensor_mul(prod[:], w3n[:], h1b[net][:])
                    nc.vector.tensor_reduce(
                        acc[:, c * dcc:(c + 1) * dcc],
                        prod.rearrange("p (a n) -> p a n", a=dcc),
                        mybir.AxisListType.X, ADD)
                if dve_cols:
                    nc.vector.tensor_add(dst[:, pe_cols:fl], acc[:, 0:dve_cols],
                                         bias_sb[:, pe_cols:fl])

            # heads: b and gate (psum [f, fb] columns)
            b3b_sb = pp.tile([128, nfb], F32, tag="b3b")
            b3gate_sb = pp.tile([128, nfb], F32, tag="b3gate")
            nc.gpsimd.dma_start(b3b_sb[:], b3b_ap[:])
            nc.gpsimd.dma_start(b3gate_sb[:], b3gate_ap[:])
            b_sb = pp.tile([128, nfb], F32, tag="bh")
            gate_sb = pp.tile([128, nfb], F32, tag="gate")
            gpre = pp.tile([128, nfb], F32, tag="gpre")
            for w3hT_ap, bias_sb, dst, net in ((w3bT_ap, b3b_sb, b_sb, 2),
                                               (w3gateT_ap, b3gate_sb, gpre, 3)):
                w3ht = sp_dve.tile([128, 2 * fl], BF16, tag="w3head")
                nc.scalar.dma_start(
                    w3ht[:], w3hT_ap.rearrange("(nb p) fl -> p nb fl", p=128))
                phd = ps_prep.tile([128, nfb], F32, tag="prep")
                for a in range(nfb):
                    for nb in range(2):
                        nc.tensor.matmul(
                            phd[:, a:a + 1],
                            w3ht[:, nb * fl + a * 128:nb * fl + (a + 1) * 128],
                            h1_sb[:, net * 2 + nb:net * 2 + nb + 1],
                            start=(nb == 0), stop=(nb == 1))
                nc.vector.tensor_add(dst[:], phd[:], bias_sb[:])
            nc.scalar.activation(gate_sb[:], gpre[:],
                                 mybir.ActivationFunctionType.Sigmoid)

            # transpose w_in/w_out to [f, z]; fold gate into w_out; sg
            w_outg = pp.tile([128, nfb * 128], BF16, tag="woutg")
            w_in_fz = pp.tile([128, nfb * 128], BF16, tag="winfz")
            sg = pp.tile([128, nfb], F32, tag="sg")
            for fb in range(nfb):
                ptr = ps_prep.tile([128, 128], BF16, tag="prep")
                nc.tensor.transpose(ptr[:], w_outT_bf[:, fb * 128:(fb + 1) * 128],
                                    eye_sb[:])
                nc.vector.tensor_scalar_mul(w_outg[:, fb * 128:(fb + 1) * 128],
                                            ptr[:], gate_sb[:, fb:fb + 1])
                pti = ps_prep.tile([128, 128], BF16, tag="prep")
                nc.tensor.transpose(pti[:], w_inT_bf[:, fb * 128:(fb + 1) * 128],
                                    eye_sb[:])
                nc.vector.tensor_copy(w_in_fz[:, fb * 128:(fb + 1) * 128], pti[:])
                # sg = sum_z w_in[f,z] * w_out[f,z] * gate[f]
                prod = wp.tile([128, 128], F32, tag="sprod")
                nc.vector.tensor_mul(prod[:], w_in_fz[:, fb * 128:(fb + 1) * 128],
                                     w_outg[:, fb * 128:(fb + 1) * 128])
                nc.vector.tensor_reduce(sg[:, fb:fb + 1], prod[:],
                                        mybir.AxisListType.X, ADD)
            sg_bf = pp.tile([128, nfb], BF16, tag="sgbf")
            nc.vector.tensor_copy(sg_bf[:], sg[:])
            b_bf = pp.tile([128, nfb], BF16, tag="bbf")
            nc.vector.tensor_copy(b_bf[:], b_sb[:])

            # ---- pack local slice into blob; AllGather ------------------
            blob_in = dp.tile([1, BLOB], BF16, tag="blobi", name="blobi")
            blob_out = dp.tile([n_cores, BLOB], BF16, tag="blobo", name="blobo",
                               addr_space="Shared")
            nc.gpsimd.dma_start(
                blob_in[0, 0:SZ_A].rearrange("(z f) -> z f", z=128),
                w_inT_bf[:])
            nc.gpsimd.dma_start(
                blob_in[0, OF_B:OF_B + SZ_B]
                .rearrange("(fb f zz) -> f fb zz", fb=nfb, f=128),
                w_outg.rearrange("p (fb zz) -> p fb zz", fb=nfb))
            nc.gpsimd.dma_start(
                blob_in[0, OF_C:OF_C + SZ_C].rearrange("(fb f) -> f fb", fb=nfb),
                sg_bf[:])
            nc.gpsimd.dma_start(
                blob_in[0, OF_D:OF_D + SZ_D].rearrange("(fb f) -> f fb", fb=nfb),
                b_bf[:])
            nc.gpsimd.collective_compute(
                "AllGather", mybir.AluOpType.bypass,
                replica_groups=[list(range(n_cores))],
                ins=[blob_in.opt()], outs=[blob_out.opt()])

            # ---- post-AG loads: global stationary tiles -----------------
            w_inT_g = pp.tile([128, f], BF16, tag="winTg")
            nc.sync.dma_start(
                w_inT_g.rearrange("z (r ff) -> z r ff", r=n_cores),
                blob_out[:, 0:SZ_A].rearrange("r (z ff) -> z r ff", z=128))
            w_outg_g = pp.tile([128, f], BF16, tag="woutgg")
            for fb in range(nfb):
                nc.scalar.dma_start(
                    w_outg_g.rearrange("ff (r fb zz) -> ff r fb zz",
                                       r=n_cores, fb=nfb)[:, :, fb, :],
                    blob_out[:, OF_B + fb * 128 * 128:
                             OF_B + (fb + 1) * 128 * 128]
                    .rearrange("r (ff zz) -> ff r zz", ff=128))
            sg_g = pp.tile([128, nfb_g], BF16, tag="sgg")
            b_gbf = pp.tile([128, nfb_g], BF16, tag="bgbf")
            for fb in range(nfb):
                nc.sync.dma_start(
                    sg_g.rearrange("ff (r fb) -> ff r fb",
                                   r=n_cores)[:, :, fb],
                    blob_out[:, OF_C + fb * 128:OF_C + (fb + 1) * 128]
                    .rearrange("r ff -> ff r"))
                nc.scalar.dma_start(
                    b_gbf.rearrange("ff (r fb) -> ff r fb",
                                    r=n_cores)[:, :, fb],
                    blob_out[:, OF_D + fb * 128:OF_D + (fb + 1) * 128]
                    .rearrange("r ff -> ff r"))
            b_g = pp.tile([128, nfb_g], F32, tag="bg")
            nc.vector.tensor_copy(b_g[:], b_gbf[:])

            # cneg = -sum_f sg / F  (global)
            sgs = pp.tile([128, 1], F32, tag="sgs")
            nc.vector.tensor_reduce(sgs[:], sg_g[:], mybir.AxisListType.X, ADD)
            csum = pp.tile([1, 1], F32, tag="csum")
            nc.gpsimd.tensor_reduce(csum[:], sgs[:], mybir.AxisListType.XYZWC,
                                    ADD)
            cneg = pp.tile([1, 1], F32, tag="cneg")
            nc.scalar.mul(cneg[:], csum[:], -1.0 / f)

            # ---- stage B: own batch shard x full F ----------------------
            for j in range(nbc):
                b0 = j * bc
                pdz = ps_dz.tile([128, bc], F32, tag="pdz")
                pt2 = ps_t2.tile([1, bc], F32, tag="pt2")
                hs = [None] * nfb_g
                h2s = [None] * nfb_g

                def emit_ph(fb):
                    ph = ps_h.tile([128, bc], F32, tag="ph")
                    nc.tensor.matmul(ph[:],
                                     w_inT_g[:, fb * 128:(fb + 1) * 128],
                                     zt_sb[:, b0:b0 + bc],
                                     start=True, stop=True)
                    h_bf = hp.tile([128, bc], BF16, tag="hbf")
                    nc.scalar.activation(h_bf[:], ph[:],
                                         mybir.ActivationFunctionType.Tanh,
                                         bias=b_g[:, fb:fb + 1])
                    h2_bf = h2p.tile([128, bc], BF16, tag="h2bf")
                    nc.vector.tensor_mul(h2_bf[:], h_bf[:], h_bf[:])
                    hs[fb] = h_bf
                    h2s[fb] = h2_bf

                def emit_acc(fb):
                    nc.tensor.matmul(pdz[:],
                                     w_outg_g[:, fb * 128:(fb + 1) * 128],
                                     hs[fb][:],
                                     start=(fb == 0), stop=(fb == nfb_g - 1))
                    nc.tensor.matmul(pt2[:], sg_g[:, fb:fb + 1], h2s[fb][:],
                                     start=(fb == 0), stop=(fb == nfb_g - 1))

                for fb in range(nfb_g):
                    emit_ph(fb)
                    if fb >= 1:
                        emit_acc(fb - 1)
                emit_acc(nfb_g - 1)

                dz_sb = wp.tile([128, bc], F32, tag="dzsb")
                nc.scalar.mul(dz_sb[:], pdz[:], 1.0 / f)
                nc.sync.dma_start(out_ap[0:Z, b0:b0 + bc], dz_sb[:])
                tr_sb = wp.tile([1, bc], F32, tag="trsb")
                nc.scalar.activation(
                    tr_sb[:], pt2[:],
                    mybir.ActivationFunctionType.Identity,
                    bias=cneg[0:1, 0:1], scale=1.0 / f)
                nc.gpsimd.dma_start(out_ap[Z:Z + 1, b0:b0 + bc], tr_sb[:])

    nc.compile()
    return nc


def host_prep(t, z_and_logpz, W1, B1, W2, B2, W3_win, b3_win,
              W3_wout, b3_wout, W3_b, b3_b, W3_gate, b3_gate,
              n_cores=N_CORES, b=B, f=F, pe_cols=PE_COLS):
    """Shard + lay out the numpy inputs into per-core in_maps."""
    fl = f // n_cores
    nfb = fl // 128
    rows = fl * Z
    rows_pe = pe_cols * 128

    dcc = DVE_CC

    def pack_nat(x):  # [rows_dve, N] -> [nch*128, dcc*N], partition-contiguous
        nch = x.shape[0] // (dcc * 128)
        return np.ascontiguousarray(
            x.reshape(nch, dcc, 128, N).transpose(0, 2, 1, 3)
            .reshape(nch * 128, dcc * N))

    def col8(x):  # [4, 256] -> [128, 8] with col = k*2 + nb
        return np.ascontiguousarray(
            np.asarray(x, np.float32).reshape(4, 2, 128).transpose(2, 0, 1)
            .reshape(128, 8))

    t_in = np.ascontiguousarray(
        np.broadcast_to(np.asarray(t, np.float32).reshape(1, 1), (128, 1)))
    parc = np.ascontiguousarray(np.concatenate(
        [col8(np.asarray(W1, np.float32)[:, :, 0]), col8(B1), col8(B2)],
        axis=1))
    # lhsT tile for h1 net: [m128, (k4, mb, n)] = W2[k4, n, mb*128+m128]
    w2tc = np.ascontiguousarray(
        np.asarray(W2, np.float32).transpose(0, 2, 1)        # [k, m, n]
        .reshape(4, 2, 128, 256).transpose(2, 0, 1, 3).reshape(128, 2048)).astype(BF)
    w3win_bf = np.asarray(W3_win, np.float32).astype(BF)
    w3wout_bf = np.asarray(W3_wout, np.float32).astype(BF)
    w3b_bf = np.asarray(W3_b, np.float32).astype(BF)
    w3gate_bf = np.asarray(W3_gate, np.float32).astype(BF)
    b3win = np.asarray(b3_win, np.float32)
    b3wout = np.asarray(b3_wout, np.float32)
    b3b = np.asarray(b3_b, np.float32)
    b3gate = np.asarray(b3_gate, np.float32)
    z = np.asarray(z_and_logpz, np.float32)[:, :Z]
    ztb = np.ascontiguousarray(z.T).astype(BF)
    eye = np.eye(128, dtype=np.float32).astype(BF)
    bl = b // n_cores

    in_maps = []
    for k in range(n_cores):
        r0 = k * rows
        f0 = k * fl
        in_maps.append({
            "t": t_in, "parc": parc, "w2tc": w2tc,
            "w3winT_sl": np.ascontiguousarray(w3win_bf[r0:r0 + rows_pe].T),
            "w3woutT_sl": np.ascontiguousarray(w3wout_bf[r0:r0 + rows_pe].T),
            "w3winN_sl": pack_nat(w3win_bf[r0 + rows_pe:r0 + rows]),
            "w3woutN_sl": pack_nat(w3wout_bf[r0 + rows_pe:r0 + rows]),
            "b3win_c": np.ascontiguousarray(
                b3win[r0:r0 + rows].reshape(fl, 128).T),
            "b3wout_c": np.ascontiguousarray(
                b3wout[r0:r0 + rows].reshape(fl, 128).T),
            "w3bT_sl": np.ascontiguousarray(w3b_bf[f0:f0 + fl].T),
            "w3gateT_sl": np.ascontiguousarray(w3gate_bf[f0:f0 + fl].T),
            "b3b_c": np.ascontiguousarray(b3b[f0:f0 + fl].reshape(nfb, 128).T),
            "b3gate_c": np.ascontiguousarray(
                b3gate[f0:f0 + fl].reshape(nfb, 128).T),
            "ztb_sl": np.ascontiguousarray(ztb[:, k * bl:(k + 1) * bl]),
            "eyeb": eye,
        })
    return in_maps


_NC_CACHE = {}


def kernel(**inputs) -> np.ndarray:
    _ensure_ntff_hook()
    from concourse import bass_utils

    key = "full"
    if key not in _NC_CACHE:
        _NC_CACHE[key] = build_module()
    nc = _NC_CACHE[key]

    in_maps = host_prep(**inputs)
    res = bass_utils.run_bass_kernel_spmd(nc, in_maps, list(range(N_CORES)))
    bl = B // N_CORES
    out = np.empty((B, Z + 1), np.float32)
    for k in range(N_CORES):
        out[k * bl:(k + 1) * bl, :] = res.results[k]["out"].T
    return out


# revision 11
# speedup vs baseline: 1.4268x; 1.0809x over previous
"""Trainium2 Bass kernel for nn_ContinousNormalizingFlowRHS.

Computes, for z in R^{B x Z} and scalar time t:
  h0 = tanh(W1*t + B1); h1 = tanh(einsum('knm,km->kn', W2, h0) + B2)
  w_in  = (W3_win  @ h1[0] + b3_win ).reshape(F, Z)
  w_out = (W3_wout @ h1[1] + b3_wout).reshape(F, Z)
  b     =  W3_b    @ h1[2] + b3_b
  gate  = sigmoid(W3_gate @ h1[3] + b3_gate)
  h = tanh(z @ w_in.T + b); dz = (h*gate) @ w_out / F
  trace = ((1-h^2)*gate) @ (sum(w_in*w_out,1)) / F
  out = concat([dz, -trace[:,None]], -1)

Strategy (8 NeuronCores, single SPMD launch):
  Phase 1 (f-sharded): the dominant cost is streaming W3_win/W3_wout
  (134 MB each in bf16), sharded row-wise across the 8 cores.  Each
  core's matvec work is split between the PE (transposed bf16 slices
  as stationary weights, h1 column as the moving operand) and the DVE
  (natural-layout slices, multiply by a partition-broadcast h1 then
  reduce along the free axis), so the HBM stream rate binds.
  Each core then packs its local slice of (w_inT, gate*w_out, sg, b)
  into a ~129 KB blob; one small AllGather replicates all slices.
  Stage B (batch-sharded): each core runs the batch matmuls for its
  OWN B/8 shard against the FULL F, accumulating dz and the trace in
  fp32 PSUM across all 16 f-blocks, and writes its final [Z+1, B/8]
  output directly -- no end-of-kernel collective.
"""

import sys
import types
import numpy as np
import ml_dtypes

BF = ml_dtypes.bfloat16

# problem sizes (hardcoded per contract)
Z = 128
N = 256
F = 2048
B = 8192
N_CORES = 8

PE_COLS = 144       # per matrix: local f-columns computed on the PE (rest DVE)
CHUNK_R = 4608      # W3 rows per streamed PE chunk ([128, 4608] bf16 tiles)
DVE_CC = 16         # f-columns per DVE chunk (2048 rows)
BC = 512            # batch columns per stage-B chunk (one PSUM bank)


def _ensure_ntff_hook():
    """run_bass_kernel_spmd(trace=True) under axon needs antenv.axon_hooks."""
    if 'antenv.axon_hooks' in sys.modules:
        return
    try:
        from trn_agent_boot.trn_boot import _ntff_profile_via_ctypes
        hook = _ntff_profile_via_ctypes('/opt/axon/libaxon_pjrt.so')
    except Exception:
        hook = None
    try:
        import antenv
    except Exception:
        return
    mod = types.ModuleType('antenv.axon_hooks')
    mod.get_axon_ntff_profile_hook = lambda: hook
    mod.set_axon_ntff_profile_hook = lambda h: None
    sys.modules['antenv.axon_hooks'] = mod
    antenv.axon_hooks = mod


def build_module(n_cores=N_CORES, b=B, f=F, pe_cols=PE_COLS, chunk_r=CHUNK_R,
                 bc=BC, debug=False):
    """Build the Bass module (SPMD program, one per core)."""
    import concourse.tile as tile
    from concourse import bacc, mybir

    F32 = mybir.dt.float32
    BF16 = mybir.dt.bfloat16
    ADD = mybir.AluOpType.add

    fl = f // n_cores            # local f count
    nfb = fl // 128              # local f blocks of 128
    nfb_g = f // 128             # global f blocks of 128
    rows_pe = pe_cols * 128      # rows of W3 handled by the PE
    dve_cols = fl - pe_cols
    rows_dve = dve_cols * 128
    n_pe_chunks = rows_pe // chunk_r
    rpc = chunk_r // 128         # w columns produced per PE chunk
    dcc = DVE_CC                 # f-columns per DVE chunk
    n_dve_chunks = dve_cols // dcc
    bl = b // n_cores            # per-core batch shard
    nbc = bl // bc               # stage-B batch chunks
    assert rows_pe % chunk_r == 0 and dve_cols % dcc == 0

    # blob layout (bf16 elements): w_inT (z,f) | w_outg (fb,f,z) | sg | b
    SZ_A = 128 * fl
    SZ_B = 128 * fl
    SZ_C = fl
    SZ_D = fl
    BLOB = SZ_A + SZ_B + SZ_C + SZ_D
    OF_B, OF_C, OF_D = SZ_A, SZ_A + SZ_B, SZ_A + SZ_B + SZ_C

    nc = bacc.Bacc("TRN2", target_bir_lowering=False, debug=debug,
                   num_devices=n_cores)

    def inp(name, shape, dt):
        return nc.dram_tensor(name, shape, dt, kind="ExternalInput").ap()

    t_ap = inp("t", [128, 1], F32)                  # t replicated
    par_ap = inp("parc", [128, 24], F32)            # w1c | b1c | b2c
    w2t_ap = inp("w2tc", [128, 2048], BF16)
    w3winT_ap = inp("w3winT_sl", [N, rows_pe], BF16)
    w3woutT_ap = inp("w3woutT_sl", [N, rows_pe], BF16)
    w3winN_ap = inp("w3winN_sl", [rows_dve // (dcc * 128) * 128, dcc * N], BF16)
    w3woutN_ap = inp("w3woutN_sl", [rows_dve // (dcc * 128) * 128, dcc * N], BF16)
    b3win_ap = inp("b3win_c", [128, fl], F32)
    b3wout_ap = inp("b3wout_c", [128, fl], F32)
    w3bT_ap = inp("w3bT_sl", [N, fl], BF16)
    w3gateT_ap = inp("w3gateT_sl", [N, fl], BF16)
    b3b_ap = inp("b3b_c", [128, nfb], F32)
    b3gate_ap = inp("b3gate_c", [128, nfb], F32)
    zt_ap = inp("ztb_sl", [128, bl], BF16)          # own batch shard only
    eye_ap = inp("eyeb", [128, 128], BF16)
    out_ap = nc.dram_tensor("out", [Z + 1, bl], F32, kind="ExternalOutput").ap()

    with tile.TileContext(nc) as tc:
        with tc.tile_pool(name="persist", bufs=1) as pp, \
             tc.tile_pool(name="sp_pe", bufs=3) as sp_pe, \
             tc.tile_pool(name="sp_dve", bufs=3) as sp_dve, \
             tc.tile_pool(name="work", bufs=3) as wp, \
             tc.tile_pool(name="hbuf", bufs=4) as hp, \
             tc.tile_pool(name="h2buf", bufs=4) as h2p, \
             tc.tile_pool(name="ps_h", bufs=2, space="PSUM") as ps_h, \
             tc.tile_pool(name="ps_dz", bufs=2, space="PSUM") as ps_dz, \
             tc.tile_pool(name="ps_t2", bufs=2, space="PSUM") as ps_t2, \
             tc.tile_pool(name="ps_prep", bufs=2, space="PSUM") as ps_prep, \
             tc.tile_pool(name="dram", bufs=1, space="DRAM") as dp:

            # ---- parameter nets (tiny) ----------------------------------
            par_sb = pp.tile([128, 24], F32, tag="parc")
            nc.gpsimd.dma_start(par_sb[:], par_ap[:])
            t_sb = pp.tile([128, 1], F32, tag="tbc")
            nc.gpsimd.dma_start(t_sb[:], t_ap[:])
            w2t_sb = pp.tile([128, 2048], BF16, tag="w2t")
            nc.gpsimd.dma_start(w2t_sb[:], w2t_ap[:])

            h0pre = pp.tile([128, 8], F32, tag="h0pre")
            nc.vector.tensor_scalar_mul(h0pre[:], par_sb[:, 0:8], t_sb[:, 0:1])
            nc.vector.tensor_add(h0pre[:], h0pre[:], par_sb[:, 8:16])
            h0_sb = pp.tile([128, 8], BF16, tag="h0")
            nc.scalar.activation(h0_sb[:], h0pre[:],
                                 mybir.ActivationFunctionType.Tanh)

            ps_h1 = ps_prep.tile([128, 8], F32, tag="prep")
            for k4 in range(4):
                for nb in range(2):
                    c = k4 * 2 + nb
                    for mb in range(2):
                        lhs = w2t_sb[:, k4 * 512 + mb * 256 + nb * 128:
                                     k4 * 512 + mb * 256 + nb * 128 + 128]
                        nc.tensor.matmul(ps_h1[:, c:c + 1], lhs,
                                         h0_sb[:, k4 * 2 + mb:k4 * 2 + mb + 1],
                                         start=(mb == 0), stop=(mb == 1))
            h1pre = pp.tile([128, 8], F32, tag="h1pre")
            h1_sb = pp.tile([128, 8], BF16, tag="h1")
            nc.vector.tensor_add(h1pre[:], ps_h1[:], par_sb[:, 16:24])
            nc.scalar.activation(h1_sb[:], h1pre[:],
                                 mybir.ActivationFunctionType.Tanh)
            # h1 -> DRAM in (net, n) order, then broadcast-load nets 0/1
            # replicated across partitions AND repeated dcc times along the
            # free dim (so the DVE multiply runs chunk-granular).
            h1_dram = dp.tile([8, 128], BF16, tag="h1d")
            nc.gpsimd.dma_start(h1_dram.rearrange("c n -> n c"), h1_sb[:])
            h1b = []
            for k4 in range(2):
                hb = pp.tile([128, dcc * N], BF16, tag=f"h1b{k4}")
                src = h1_dram.rearrange("c n -> (c n)")[k4 * N:(k4 + 1) * N]
                src = src.unsqueeze(0).unsqueeze(0)
                nc.gpsimd.dma_start(hb[:], src.broadcast_to([128, dcc, N]))
                h1b.append(hb)

            # stage-B constants, loaded early
            zt_sb = pp.tile([128, bl], BF16, tag="zt")
            nc.scalar.dma_start(zt_sb[:], zt_ap[:])
            eye_sb = pp.tile([128, 128], BF16, tag="eye")
            nc.gpsimd.dma_start(eye_sb[:], eye_ap[:])
            b3win_sb = pp.tile([128, fl], F32, tag="b3win")
            b3wout_sb = pp.tile([128, fl], F32, tag="b3wout")
            nc.scalar.dma_start(b3win_sb[:], b3win_ap[:])
            nc.scalar.dma_start(b3wout_sb[:], b3wout_ap[:])

            # ---- phase 1: sharded matvecs, split across PE and DVE ------
            # Emission is round-robin across the PE-path and DVE-path chunks
            # so neither engine queue is head-of-line blocked by the other
            # path's drains, and the HBM stream stays saturated.
            w_inT_bf = pp.tile([128, fl], BF16, tag="winT")
            w_outT_bf = pp.tile([128, fl], BF16, tag="woutT")
            daccs = {}
            for net, nm in ((0, "win"), (1, "wout")):
                daccs[net] = pp.tile([128, max(dve_cols, 1)], F32,
                                     tag=f"dacc{net}", name=f"dacc{nm}")

            def emit_pe_chunk(w3T_ap, bias_sb, dst, net, c):
                tiles = []
                for nb in range(2):
                    w3t = sp_pe.tile([128, chunk_r], BF16, tag="w3chunk")
                    nc.sync.dma_start(
                        w3t[:], w3T_ap[nb * 128:(nb + 1) * 128,
                                       c * chunk_r:(c + 1) * chunk_r])
                    tiles.append(w3t)
                pw = ps_prep.tile([128, rpc], F32, tag="prep")
                for a in range(rpc):
                    for nb in range(2):
                        nc.tensor.matmul(
                            pw[:, a:a + 1],
                            tiles[nb][:, a * 128:(a + 1) * 128],
                            h1_sb[:, net * 2 + nb:net * 2 + nb + 1],
                            start=(nb == 0), stop=(nb == 1))
                nc.vector.tensor_add(dst[:, c * rpc:(c + 1) * rpc], pw[:],
                                     bias_sb[:, c * rpc:(c + 1) * rpc])

            def emit_dve_chunk(w3N_ap, bias_sb, dst, net, c):
                w3n = sp_dve.tile([128, dcc * N], BF16, tag="w3nat")
                nc.scalar.dma_start(w3n[:], w3N_ap[c * 128:(c + 1) * 128, :])
                prod = wp.tile([128, dcc * N], BF16, tag="prod")
                nc.vector.tensor_mul(prod[:], w3n[:], h1b[net][:])
                nc.vector.tensor_reduce(
                    daccs[net][:, c * dcc:(c + 1) * dcc],
                    prod.rearrange("p (a n) -> p a n", a=dcc),
                    mybir.AxisListType.X, ADD)
                if c == n_dve_chunks - 1:
                    nc.vector.tensor_add(dst[:, pe_cols:fl],
                                         daccs[net][:, 0:dve_cols],
                                         bias_sb[:, pe_cols:fl])

            # alternate matrices so both dst tiles complete about together
            pe_units = []
            dve_units = []
            for c in range(n_pe_chunks):
                pe_units.append((w3winT_ap, b3win_sb, w_inT_bf, 0, c))
                pe_units.append((w3woutT_ap, b3wout_sb, w_outT_bf, 1, c))
            for c in range(n_dve_chunks):
                dve_units.append((w3winN_ap, b3win_sb, w_inT_bf, 0, c))
                dve_units.append((w3woutN_ap, b3wout_sb, w_outT_bf, 1, c))
            # heads first: b and gate (psum [f, fb] columns) -- gate gates
            # the per-fb transposes below, so compute it early.
            b3b_sb = pp.tile([128, nfb], F32, tag="b3b")
            b3gate_sb = pp.tile([128, nfb], F32, tag="b3gate")
            nc.gpsimd.dma_start(b3b_sb[:], b3b_ap[:])
            nc.gpsimd.dma_start(b3gate_sb[:], b3gate_ap[:])
            b_sb = pp.tile([128, nfb], F32, tag="bh")
            gate_sb = pp.tile([128, nfb], F32, tag="gate")
            gpre = pp.tile([128, nfb], F32, tag="gpre")
            for w3hT_ap, bias_sb, dst, net in ((w3bT_ap, b3b_sb, b_sb, 2),
                                               (w3gateT_ap, b3gate_sb, gpre, 3)):
                w3ht = sp_dve.tile([128, 2 * fl], BF16, tag="w3head")
                nc.scalar.dma_start(
                    w3ht[:], w3hT_ap.rearrange("(nb p) fl -> p nb fl", p=128))
                phd = ps_prep.tile([128, nfb], F32, tag="prep")
                for a in range(nfb):
                    for nb in range(2):
                        nc.tensor.matmul(
                            phd[:, a:a + 1],
                            w3ht[:, nb * fl + a * 128:nb * fl + (a + 1) * 128],
                            h1_sb[:, net * 2 + nb:net * 2 + nb + 1],
                            start=(nb == 0), stop=(nb == 1))
                nc.vector.tensor_add(dst[:], phd[:], bias_sb[:])
            nc.scalar.activation(gate_sb[:], gpre[:],
                                 mybir.ActivationFunctionType.Sigmoid)
            b_bf = pp.tile([128, nfb], BF16, tag="bbf")
            nc.vector.tensor_copy(b_bf[:], b_sb[:])

            blob_in = dp.tile([1, BLOB], BF16, tag="blobi", name="blobi")
            blob_out = dp.tile([n_cores, BLOB], BF16, tag="blobo", name="blobo",
                               addr_space="Shared")
            w_outg = pp.tile([128, nfb * 128], BF16, tag="woutg")
            w_in_fz = pp.tile([128, nfb * 128], BF16, tag="winfz")
            sg = pp.tile([128, nfb], F32, tag="sg")
            sg_bf = pp.tile([128, nfb], BF16, tag="sgbf")

            def emit_fb_group(fb):
                # transpose w_in/w_out block to [f, z]; fold gate into w_out;
                # sg; then stream this block's slice of the blob out.
                ptr = ps_prep.tile([128, 128], BF16, tag="prep")
                nc.tensor.transpose(ptr[:], w_outT_bf[:, fb * 128:(fb + 1) * 128],
                                    eye_sb[:])
                nc.vector.tensor_scalar_mul(w_outg[:, fb * 128:(fb + 1) * 128],
                                            ptr[:], gate_sb[:, fb:fb + 1])
                pti = ps_prep.tile([128, 128], BF16, tag="prep")
                nc.tensor.transpose(pti[:], w_inT_bf[:, fb * 128:(fb + 1) * 128],
                                    eye_sb[:])
                nc.vector.tensor_copy(w_in_fz[:, fb * 128:(fb + 1) * 128], pti[:])
                # sg = sum_z w_in[f,z] * w_out[f,z] * gate[f]
                prod = wp.tile([128, 128], F32, tag="sprod")
                nc.vector.tensor_mul(prod[:], w_in_fz[:, fb * 128:(fb + 1) * 128],
                                     w_outg[:, fb * 128:(fb + 1) * 128])
                nc.vector.tensor_reduce(sg[:, fb:fb + 1], prod[:],
                                        mybir.AxisListType.X, ADD)
                nc.gpsimd.dma_start(
                    blob_in[0, 0:SZ_A]
                    .rearrange("(z f) -> z f", z=128)[:, fb * 128:(fb + 1) * 128],
                    w_inT_bf[:, fb * 128:(fb + 1) * 128])
                nc.gpsimd.dma_start(
                    blob_in[0, OF_B + fb * 128 * 128:OF_B + (fb + 1) * 128 * 128]
                    .rearrange("(f zz) -> f zz", f=128),
                    w_outg[:, fb * 128:(fb + 1) * 128])

            # interleave PE/DVE chunks; keep the last DVE chunks after the
            # fb0 transpose group so fb0's blob slice streams out early.
            npe, ndve = len(pe_units), len(dve_units)
            dve_hold = 4
            di = 0
            for pi in range(npe):
                emit_pe_chunk(*pe_units[pi])
                dt = min((pi + 1) * ndve // npe, ndve - dve_hold)
                while di < dt:
                    emit_dve_chunk(*dve_units[di])
                    di += 1
            emit_fb_group(0)          # fb0 = cols [0,128) -- PE part complete
            while di < ndve:
                emit_dve_chunk(*dve_units[di])
                di += 1
            emit_fb_group(1)          # fb1 = cols [128,256)
            nc.vector.tensor_copy(sg_bf[:], sg[:])
            nc.gpsimd.dma_start(
                blob_in[0, OF_C:OF_C + SZ_C].rearrange("(fb f) -> f fb", fb=nfb),
                sg_bf[:])
            nc.gpsimd.dma_start(
                blob_in[0, OF_D:OF_D + SZ_D].rearrange("(fb f) -> f fb", fb=nfb),
                b_bf[:])
            nc.gpsimd.collective_compute(
                "AllGather", mybir.AluOpType.bypass,
                replica_groups=[list(range(n_cores))],
                ins=[blob_in.opt()], outs=[blob_out.opt()])

            # ---- post-AG loads: global stationary tiles -----------------
            w_inT_g = pp.tile([128, f], BF16, tag="winTg")
            nc.sync.dma_start(
                w_inT_g.rearrange("z (r ff) -> z r ff", r=n_cores),
                blob_out[:, 0:SZ_A].rearrange("r (z ff) -> z r ff", z=128))
            w_outg_g = pp.tile([128, f], BF16, tag="woutgg")
            for fb in range(nfb):
                nc.scalar.dma_start(
                    w_outg_g.rearrange("ff (r fb zz) -> ff r fb zz",
                                       r=n_cores, fb=nfb)[:, :, fb, :],
                    blob_out[:, OF_B + fb * 128 * 128:
                             OF_B + (fb + 1) * 128 * 128]
                    .rearrange("r (ff zz) -> ff r zz", ff=128))
            sg_g = pp.tile([128, nfb_g], BF16, tag="sgg")
            b_gbf = pp.tile([128, nfb_g], BF16, tag="bgbf")
            for fb in range(nfb):
                nc.sync.dma_start(
                    sg_g.rearrange("ff (r fb) -> ff r fb",
                                   r=n_cores)[:, :, fb],
                    blob_out[:, OF_C + fb * 128:OF_C + (fb + 1) * 128]
                    .rearrange("r ff -> ff r"))
                nc.scalar.dma_start(
                    b_gbf.rearrange("ff (r fb) -> ff r fb",
                                    r=n_cores)[:, :, fb],
                    blob_out[:, OF_D + fb * 128:OF_D + (fb + 1) * 128]
                    .rearrange("r ff -> ff r"))
            b_g = pp.tile([128, nfb_g], F32, tag="bg")
            nc.vector.tensor_copy(b_g[:], b_gbf[:])

            # cneg = -sum_f sg / F  (global)
            sgs = pp.tile([128, 1], F32, tag="sgs")
            nc.vector.tensor_reduce(sgs[:], sg_g[:], mybir.AxisListType.X, ADD)
            csum = pp.tile([1, 1], F32, tag="csum")
            nc.gpsimd.tensor_reduce(csum[:], sgs[:], mybir.AxisListType.XYZWC,
                                    ADD)
            cneg = pp.tile([1, 1], F32, tag="cneg")
            nc.scalar.mul(cneg[:], csum[:], -1.0 / f)

            # ---- stage B: own batch shard x full F ----------------------
            for j in range(nbc):
                b0 = j * bc
                pdz = ps_dz.tile([128, bc], F32, tag="pdz")
                pt2 = ps_t2.tile([1, bc], F32, tag="pt2")
                hs = [None] * nfb_g
                h2s = [None] * nfb_g

                def emit_ph(fb):
                    ph = ps_h.tile([128, bc], F32, tag="ph")
                    nc.tensor.matmul(ph[:],
                                     w_inT_g[:, fb * 128:(fb + 1) * 128],
                                     zt_sb[:, b0:b0 + bc],
                                     start=True, stop=True)
                    h_bf = hp.tile([128, bc], BF16, tag="hbf")
                    nc.scalar.activation(h_bf[:], ph[:],
                                         mybir.ActivationFunctionType.Tanh,
                                         bias=b_g[:, fb:fb + 1])
                    h2_bf = h2p.tile([128, bc], BF16, tag="h2bf")
                    nc.vector.tensor_mul(h2_bf[:], h_bf[:], h_bf[:])
                    hs[fb] = h_bf
                    h2s[fb] = h2_bf

                def emit_acc(fb):
                    nc.tensor.matmul(pdz[:],
                                     w_outg_g[:, fb * 128:(fb + 1) * 128],
                                     hs[fb][:],
                                     start=(fb == 0), stop=(fb == nfb_g - 1))
                    nc.tensor.matmul(pt2[:], sg_g[:, fb:fb + 1], h2s[fb][:],
                                     start=(fb == 0), stop=(fb == nfb_g - 1))

                for fb in range(nfb_g):
                    emit_ph(fb)
                    if fb >= 1:
                        emit_acc(fb - 1)
                emit_acc(nfb_g - 1)

                dz_sb = wp.tile([128, bc], F32, tag="dzsb")
                nc.scalar.mul(dz_sb[:], pdz[:], 1.0 / f)
                nc.sync.dma_start(out_ap[0:Z, b0:b0 + bc], dz_sb[:])
                tr_sb = wp.tile([1, bc], F32, tag="trsb")
                nc.scalar.activation(
                    tr_sb[:], pt2[:],
                    mybir.ActivationFunctionType.Identity,
                    bias=cneg[0:1, 0:1], scale=1.0 / f)
                nc.gpsimd.dma_start(out_ap[Z:Z + 1, b0:b0 + bc], tr_sb[:])

    nc.compile()
    return nc


def host_prep(t, z_and_logpz, W1, B1, W2, B2, W3_win, b3_win,
              W3_wout, b3_wout, W3_b, b3_b, W3_gate, b3_gate,
              n_cores=N_CORES, b=B, f=F, pe_cols=PE_COLS):
    """Shard + lay out the numpy inputs into per-core in_maps."""
    fl = f // n_cores
    nfb = fl // 128
    rows = fl * Z
    rows_pe = pe_cols * 128

    dcc = DVE_CC

    def pack_nat(x):  # [rows_dve, N] -> [nch*128, dcc*N], partition-contiguous
        nch = x.shape[0] // (dcc * 128)
        return np.ascontiguousarray(
            x.reshape(nch, dcc, 128, N).transpose(0, 2, 1, 3)
            .reshape(nch * 128, dcc * N))

    def col8(x):  # [4, 256] -> [128, 8] with col = k*2 + nb
        return np.ascontiguousarray(
            np.asarray(x, np.float32).reshape(4, 2, 128).transpose(2, 0, 1)
            .reshape(128, 8))

    t_in = np.ascontiguousarray(
        np.broadcast_to(np.asarray(t, np.float32).reshape(1, 1), (128, 1)))
    parc = np.ascontiguousarray(np.concatenate(
        [col8(np.asarray(W1, np.float32)[:, :, 0]), col8(B1), col8(B2)],
        axis=1))
    # lhsT tile for h1 net: [m128, (k4, mb, n)] = W2[k4, n, mb*128+m128]
    w2tc = np.ascontiguousarray(
        np.asarray(W2, np.float32).transpose(0, 2, 1)        # [k, m, n]
        .reshape(4, 2, 128, 256).transpose(2, 0, 1, 3).reshape(128, 2048)).astype(BF)
    w3win_bf = np.asarray(W3_win, np.float32).astype(BF)
    w3wout_bf = np.asarray(W3_wout, np.float32).astype(BF)
    w3b_bf = np.asarray(W3_b, np.float32).astype(BF)
    w3gate_bf = np.asarray(W3_gate, np.float32).astype(BF)
    b3win = np.asarray(b3_win, np.float32)
    b3wout = np.asarray(b3_wout, np.float32)
    b3b = np.asarray(b3_b, np.float32)
    b3gate = np.asarray(b3_gate, np.float32)
    z = np.asarray(z_and_logpz, np.float32)[:, :Z]
    ztb = np.ascontiguousarray(z.T).astype(BF)
    eye = np.eye(128, dtype=np.float32).astype(BF)
    bl = b // n_cores

    in_maps = []
    for k in range(n_cores):
        r0 = k * rows
        f0 = k * fl
        in_maps.append({
            "t": t_in, "parc": parc, "w2tc": w2tc,
            "w3winT_sl": np.ascontiguousarray(w3win_bf[r0:r0 + rows_pe].T),
            "w3woutT_sl": np.ascontiguousarray(w3wout_bf[r0:r0 + rows_pe].T),
            "w3winN_sl": pack_nat(w3win_bf[r0 + rows_pe:r0 + rows]),
            "w3woutN_sl": pack_nat(w3wout_bf[r0 + rows_pe:r0 + rows]),
            "b3win_c": np.ascontiguousarray(
                b3win[r0:r0 + rows].reshape(fl, 128).T),
            "b3wout_c": np.ascontiguousarray(
                b3wout[r0:r0 + rows].reshape(fl, 128).T),
            "w3bT_sl": np.ascontiguousarray(w3b_bf[f0:f0 + fl].T),
            "w3gateT_sl": np.ascontiguousarray(w3gate_bf[f0:f0 + fl].T),
            "b3b_c": np.ascontiguousarray(b3b[f0:f0 + fl].reshape(nfb, 128).T),
            "b3gate_c": np.ascontiguousarray(
                b3gate[f0:f0 + fl].reshape(nfb, 128).T),
            "ztb_sl": np.ascontiguousarray(ztb[:, k * bl:(k + 1) * bl]),
            "eyeb": eye,
        })
    return in_maps


_NC_CACHE = {}


def kernel(**inputs) -> np.ndarray:
    _ensure_ntff_hook()
    from concourse import bass_utils

    key = "full"
    if key not in _NC_CACHE:
        _NC_CACHE[key] = build_module()
    nc = _NC_CACHE[key]

    in_maps = host_prep(**inputs)
    res = bass_utils.run_bass_kernel_spmd(nc, in_maps, list(range(N_CORES)))
    bl = B // N_CORES
    out = np.empty((B, Z + 1), np.float32)
    for k in range(N_CORES):
        out[k * bl:(k + 1) * bl, :] = res.results[k]["out"].T
    return out
